# revision 19
# baseline (speedup 1.0000x reference)
"""DGCNN aggregation (3 EdgeConv layers) as a Bass/Tile kernel on 8 TRN2 NeuronCores.

Sharding: data-parallel over batch (B=8 -> 1 point cloud per core).
BatchNorm stats are AllReduced across cores.

Key structure per layer (per core, x: (C, N) in SBUF):
  - knn: pd/2 = (-xx_i + 2 x_i.x_j - xx_j)/2 folded into ONE fp32 PE matmul
    using two extra contraction rows; monotone transform => exact same top-k
    selection as the reference.
  - top-20 per row: 3 rounds of DVE max / max_index / match_replace (exact).
  - conv1 by linearity: y1[:, n, j] = (A x)[:, j] + ((B - A) x)[:, n]
    (A = W[:, :C], B = W[:, C:]) -> two small matmuls + gpsimd ap_gather of
    the 64/128-channel point table along the edge list.
  - BN: per-channel sum via stt accum_out, sumsq via ACT Square accum_out,
    AllReduce(sum, sumsq), affine = ACT scale/bias; LReLU = max(0.2x, x) (stt).
  - conv2 (where present): K=64 matmul over the 40960-edge tensor; max over k
    commutes with the (positive-scale) BN affine + LReLU, so only
    max_k(conv2 psum) is kept.
"""

import numpy as np
from contextlib import ExitStack

import concourse.bass as bass
import concourse.bacc as bacc
import concourse.tile as tile
import concourse.mybir as mybir

dt = mybir.dt
Alu = mybir.AluOpType
Act = mybir.ActivationFunctionType

KNN = 20
EPS = 1e-5
NEG = 0.2
P = 128          # points per tile (partition dim)
B_FULL = 8       # batch size of the full problem
N_FULL = 2048    # points per cloud
MMF = 512        # matmul moving free dim (fp32 max)

# layer specs: (conv1: Cin->Cmid, conv2: Cmid->Cout or None)
# conv1 weights W (Cmid, 2*Cin); conv2 weights (Cout, Cmid)
LAYERS = [
    dict(cin=3,  cmid=64,  w1="W1", g1="g1", b1="b1",
         cout=64, w2="W2", g2="g2", b2="b2", out="o1"),
    dict(cin=64, cmid=64,  w1="W3", g1="g3", b1="b3",
         cout=64, w2="W4", g2="g4", b2="b4", out="o2"),
    dict(cin=64, cmid=128, w1="W5", g1="g5", b1="b5",
         cout=None, w2=None, g2=None, b2=None, out="o3"),
]


def _bn_affine(nc, pool, gst, gv, bv, count, cmid):
    """From gst=(cmid,2)=[sum,sumsq] allreduced -> alpha=(cmid,1), beta=(cmid,1)."""
    mean = pool.tile([cmid, 1], dt.float32, tag="bn_mean")
    ex2 = pool.tile([cmid, 1], dt.float32, tag="bn_ex2")
    var = pool.tile([cmid, 1], dt.float32, tag="bn_var")
    sd = pool.tile([cmid, 1], dt.float32, tag="bn_sd")
    rstd = pool.tile([cmid, 1], dt.float32, tag="bn_rstd")
    alpha = pool.tile([cmid, 1], dt.float32, tag="bn_alpha")
    beta = pool.tile([cmid, 1], dt.float32, tag="bn_beta")
    inv = 1.0 / count
    nc.vector.tensor_scalar(mean[:], gst[:, 0:1], inv, None, Alu.mult)
    nc.vector.tensor_scalar(ex2[:], gst[:, 1:2], inv, None, Alu.mult)
    # var = ex2 - mean^2
    nc.vector.tensor_mul(var[:], mean[:], mean[:])
    nc.vector.tensor_sub(var[:], ex2[:], var[:])
    nc.vector.tensor_scalar(sd[:], var[:], EPS, None, Alu.add)
    nc.scalar.activation(sd[:], sd[:], Act.Sqrt)
    nc.vector.reciprocal(rstd[:], sd[:])
    nc.vector.tensor_mul(alpha[:], gv[:], rstd[:])
    # beta = b - mean * alpha
    nc.vector.tensor_mul(beta[:], mean[:], alpha[:])
    nc.vector.tensor_sub(beta[:], bv[:], beta[:])
    return alpha, beta


def emit_dgcnn(tc, ins, outs, n_points, n_cores, ctx):
    """ins/outs: dicts name -> AP (DRAM). Emits full 3-layer DGCNN for one core."""
    nc = tc.nc
    N = n_points
    NT = N // P
    E = P * KNN            # edges per point-tile
    S = E // 16            # wrapped idx cols per tile
    MMF = min(512, N)      # matmul moving-dim chunk
    NC4 = N // MMF         # matmul chunks over N
    count = float(n_cores * N * KNN)
    groups_all = list(range(n_cores))
    cc_space = {"addr_space": "Shared"} if n_cores > 4 else {}

    # ---------------- pools ----------------
    tabs = ctx.enter_context(tc.tile_pool(name="tabs", bufs=1))       # per-layer tables
    pdp = ctx.enter_context(tc.tile_pool(name="pd", bufs=2))          # pd tiles (128,N)
    gp = ctx.enter_context(tc.tile_pool(name="gath", bufs=2))         # gathered (c,E)
    yp = ctx.enter_context(tc.tile_pool(name="ybuf", bufs=2))         # y tiles (c,E)
    tiny = ctx.enter_context(tc.tile_pool(name="tiny", bufs=4))       # v8/idx
    stat = ctx.enter_context(tc.tile_pool(name="stat", bufs=1))       # stats/alpha/beta
    keep = ctx.enter_context(tc.tile_pool(name="keep", bufs=2))       # x_cur / kmax
    psA = ctx.enter_context(tc.tile_pool(name="psA", bufs=3, space="PSUM"))   # (128,512)
    psC = ctx.enter_context(tc.tile_pool(name="psC", bufs=1, space="PSUM"))   # (cout,E)
    drp = ctx.enter_context(tc.tile_pool(name="drp", bufs=1, space="DRAM"))

    x_cur = None  # SBUF AP (C, N) of current layer input

    for li, L in enumerate(LAYERS):
        cin, cmid, cout = L["cin"], L["cmid"], L["cout"]
        has2 = cout is not None
        cg = cmid // 16  # gather groups

        # ---------------- tables ----------------
        # transposed conv1 weights: WT (2*cin, cmid)
        wta = tabs.tile([cin, cmid], dt.float32, tag="wta")
        wtb = tabs.tile([cin, cmid], dt.float32, tag="wtb")
        wtr = ins[L["w1"]].rearrange("o i -> i o")
        nc.sync.dma_start(out=wta[:], in_=wtr[0:cin, :])
        nc.sync.dma_start(out=wtb[:], in_=wtr[cin:2 * cin, :])
        at = wta[:]
        dtl = tabs.tile([cin, cmid], dt.float32, tag="dt")
        nc.vector.tensor_sub(dtl[:], wtb[:], at)
        gv1 = stat.tile([cmid, 1], dt.float32, tag="gv1")
        bv1 = stat.tile([cmid, 1], dt.float32, tag="bv1")
        nc.sync.dma_start(out=gv1[:], in_=ins[L["g1"]])
        nc.sync.dma_start(out=bv1[:], in_=ins[L["b1"]])
        if has2:
            w2t = tabs.tile([cmid, cout], dt.float32, tag="w2t")
            nc.sync.dma_start(out=w2t[:], in_=ins[L["w2"]].rearrange("o i -> i o"))
            gv2 = stat.tile([cout, 1], dt.float32, tag="gv2")
            bv2 = stat.tile([cout, 1], dt.float32, tag="bv2")
            nc.sync.dma_start(out=gv2[:], in_=ins[L["g2"]])
            nc.sync.dma_start(out=bv2[:], in_=ins[L["b2"]])

        # tileA (lhsT src) = [x; -xx/2; -1]; tileB (rhs src) = [x; 1; xx/2]
        ta = tabs.tile([cin + 2, N], dt.float32, tag="ta")
        tb = tabs.tile([cin + 2, N], dt.float32, tag="tb")
        if li == 0:
            nc.sync.dma_start(out=ta[0:cin, :], in_=ins["x"])
            nc.sync.dma_start(out=tb[0:cin, :], in_=ins["x"])
        else:
            nc.scalar.copy(ta[0:cin, :], x_cur)
            nc.scalar.copy(tb[0:cin, :], x_cur)
        xv = ta[0:cin, :]

        # engine writes must start at partition 0/32/64/96 -> build the two
        # augmented rows in partition-0 scratch, then SBUF->SBUF DMA them in.
        crow = tabs.tile([1, N], dt.float32, tag="crow")   # const +-1
        xxh = tabs.tile([1, N], dt.float32, tag="xxh")     # +xx/2
        xxn = tabs.tile([1, N], dt.float32, tag="xxn")     # -xx/2
        nc.vector.memset(crow[:], 1.0)
        nc.sync.dma_start(out=tb[cin:cin + 1, :], in_=crow[:])
        nc.vector.memset(crow[:], -1.0)
        nc.sync.dma_start(out=ta[cin + 1:cin + 2, :], in_=crow[:])

        xsq = tabs.tile([cin, N], dt.float32, tag="xsq")
        nc.scalar.activation(xsq[:], xv, Act.Square)
        half = tabs.tile([cin, 1], dt.float32, tag="half")
        nc.vector.memset(half[:], 0.5)
        for c in range(NC4):
            ck = slice(c * MMF, (c + 1) * MMF)
            ps = psA.tile([P, MMF], dt.float32, tag="psA")
            nc.tensor.matmul(ps[0:1, :], half[:], xsq[:, ck])
            nc.scalar.activation(xxh[0:1, ck], ps[0:1, :], Act.Copy)
            nc.scalar.mul(xxn[0:1, ck], ps[0:1, :], -1.0)
        nc.sync.dma_start(out=tb[cin + 1:cin + 2, :], in_=xxh[:])
        nc.sync.dma_start(out=ta[cin:cin + 1, :], in_=xxn[:])

        # point tables u = A x, w = (B - A) x  (cmid, N)
        u_sb = tabs.tile([cmid, N], dt.float32, tag="u_sb")
        w_sb = tabs.tile([cmid, N], dt.float32, tag="w_sb")
        for c in range(NC4):
            ck = slice(c * MMF, (c + 1) * MMF)
            ps = psA.tile([P, MMF], dt.float32, tag="psA")
            nc.tensor.matmul(ps[0:cmid, :], at, xv[:, ck])
            nc.scalar.activation(u_sb[:, ck], ps[0:cmid, :], Act.Copy)
            ps = psA.tile([P, MMF], dt.float32, tag="psA")
            nc.tensor.matmul(ps[0:cmid, :], dtl[:], xv[:, ck])
            nc.scalar.activation(w_sb[:, ck], ps[0:cmid, :], Act.Copy)

        # persistent per-layer: wrapped idx + per-tile stat slots + kmax
        idxw = tabs.tile([cmid, S * NT], dt.int16, tag="idxw")
        didx = drp.tile([P, KNN * NT], dt.int16, tag="didx")
        sum1 = stat.tile([cmid, NT * 8], dt.float32, tag="sum1")
        sq1 = stat.tile([cmid, NT], dt.float32, tag="sq1")
        ckm = cout if has2 else cmid
        kmax = keep.tile([ckm, N], dt.float32, tag="kmax")
        if has2:
            sum2 = stat.tile([cout, NT], dt.float32, tag="sum2")
            sq2 = stat.tile([cout, NT], dt.float32, tag="sq2")

        # ---------------- pass 1: knn + topk + gather + stats ----------------
        for m in range(NT):
            pm = slice(m * P, (m + 1) * P)
            pd = pdp.tile([P, N], dt.float32, tag="pd")
            for c in range(NC4):
                ck = slice(c * MMF, (c + 1) * MMF)
                ps = psA.tile([P, MMF], dt.float32, tag="psA")
                nc.tensor.matmul(ps[:], ta[:, pm], tb[:, ck])
                nc.scalar.activation(pd[:, ck], ps[:], Act.Copy)

            idx = tiny.tile([P, 24], dt.uint16, tag="idx")
            for r in range(3):
                v8 = tiny.tile([P, 8], dt.float32, tag="v8")
                nc.vector.max(out=v8[:], in_=pd[:])
                nc.vector.max_index(out=idx[:, 8 * r:8 * r + 8], in_max=v8[:],
                                    in_values=pd[:])
                if r < 2:
                    nc.vector.match_replace(out=pd[:], in_to_replace=v8[:],
                                            in_values=pd[:], imm_value=-3.0e38)

            # idx wrap: SBUF (P,20)u16 -> DRAM -> SBUF (cmid, S) int16 wrapped
            dk = slice(m * KNN, (m + 1) * KNN)
            nc.sync.dma_start(out=didx[:, dk], in_=idx[:, 0:KNN].bitcast(dt.int16))
            # edge order e = g*(20*16) + k*16 + q  (p = 16g + q)
            # => idxw[q, g*20 + k] = idx[16g+q, k]; both DMA sides contiguous
            src = didx[:, dk].rearrange("(g q) k -> q g k", g=8)
            for gi in range(cg):
                nc.sync.dma_start(
                    out=idxw[16 * gi:16 * (gi + 1), m * S:(m + 1) * S], in_=src)

            g_t = gp.tile([cmid, E], dt.float32, tag="g_t")
            nc.gpsimd.ap_gather(out_ap=g_t[:], in_ap=u_sb[:],
                                idxs_ap=idxw[0:cmid, m * S:(m + 1) * S],
                                channels=cmid, num_elems=N, d=1, num_idxs=E)
            y_t = yp.tile([cmid, E], dt.float32, tag="y_t")
            GE = 16 * KNN  # edges per 16-point group
            for g in range(8):
                ge = slice(g * GE, (g + 1) * GE)
                wb = w_sb[:, m * P + 16 * g:m * P + 16 * (g + 1)]
                wb = wb.unsqueeze(1).to_broadcast([cmid, KNN, 16])
                nc.vector.scalar_tensor_tensor(
                    out=y_t[:, ge].rearrange("c (k q) -> c k q", q=16),
                    in0=g_t[:, ge].rearrange("c (k q) -> c k q", q=16),
                    scalar=0.0, in1=wb, op0=Alu.add, op1=Alu.add,
                    accum_out=sum1[:, 8 * m + g:8 * m + g + 1])
                if not has2:
                    nc.vector.tensor_reduce(
                        out=kmax[:, m * P + 16 * g:m * P + 16 * (g + 1)],
                        in_=y_t[:, ge].rearrange("c (k q) -> c q k", q=16),
                        axis=mybir.AxisListType.X, op=Alu.max)
            # in-place square (destroys y_t) for sumsq
            nc.scalar.activation(y_t[:], y_t[:], Act.Square,
                                 accum_out=sq1[:, m:m + 1])

        # ---------------- BN1 allreduce ----------------
        st1 = stat.tile([cmid, 2], dt.float32, tag="st1")
        nc.vector.tensor_reduce(out=st1[:, 0:1], in_=sum1[:],
                                axis=mybir.AxisListType.X, op=Alu.add)
        nc.vector.tensor_reduce(out=st1[:, 1:2], in_=sq1[:],
                                axis=mybir.AxisListType.X, op=Alu.add)
        cc_in1 = nc.dram_tensor(f"cc_in1_{li}", [cmid, 2], dt.float32)
        cc_out1 = nc.dram_tensor(f"cc_out1_{li}", [cmid, 2], dt.float32,
                                 **cc_space)
        nc.sync.dma_start(out=cc_in1.ap(), in_=st1[:])
        nc.gpsimd.collective_compute(
            "AllReduce", Alu.add, replica_groups=[groups_all],
            ins=[cc_in1.ap()], outs=[cc_out1.ap()])
        gst1 = stat.tile([cmid, 2], dt.float32, tag="gst1")
        nc.sync.dma_start(out=gst1[:], in_=cc_out1.ap())
        al1, be1 = _bn_affine(nc, stat, gst1, gv1, bv1, count, cmid)

        if has2:
            # ---------------- pass 2: regather + BN+LReLU + conv2 ----------------
            for m in range(NT):
                pm = slice(m * P, (m + 1) * P)
                g2 = gp.tile([cmid, E], dt.float32, tag="g_t")
                nc.gpsimd.ap_gather(out_ap=g2[:], in_ap=u_sb[:],
                                    idxs_ap=idxw[0:cmid, m * S:(m + 1) * S],
                                    channels=cmid, num_elems=N, d=1, num_idxs=E)
                s_t = yp.tile([cmid, E], dt.float32, tag="y_t")
                GE = 16 * KNN
                for g in range(8):
                    ge = slice(g * GE, (g + 1) * GE)
                    wb = w_sb[:, m * P + 16 * g:m * P + 16 * (g + 1)]
                    wb = wb.unsqueeze(1).to_broadcast([cmid, KNN, 16])
                    nc.vector.scalar_tensor_tensor(
                        out=s_t[:, ge].rearrange("c (k q) -> c k q", q=16),
                        in0=g2[:, ge].rearrange("c (k q) -> c k q", q=16),
                        scalar=0.0, in1=wb, op0=Alu.add, op1=Alu.add)
                # BN affine (ACT) then LReLU (stt max(0.2x, x))
                nc.scalar.activation(s_t[:], s_t[:], Act.Identity,
                                     bias=be1[:], scale=al1[:])
                nc.vector.scalar_tensor_tensor(
                    out=s_t[:], in0=s_t[:], scalar=NEG, in1=s_t[:],
                    op0=Alu.mult, op1=Alu.max)
                ps2 = psC.tile([cout, E], dt.float32, tag="psC")
                for c in range(E // MMF):
                    ck = slice(c * MMF, (c + 1) * MMF)
                    nc.tensor.matmul(ps2[:, ck], w2t[:], s_t[:, ck])
                for g in range(8):
                    ge = slice(g * GE, (g + 1) * GE)
                    nc.vector.tensor_reduce(
                        out=kmax[:, m * P + 16 * g:m * P + 16 * (g + 1)],
                        in_=ps2[:, ge].rearrange("c (k q) -> c q k", q=16),
                        axis=mybir.AxisListType.X, op=Alu.max)
                # stats of conv2 output (scratch reuse of g2 / s_t)
                nc.scalar.activation(g2[0:cout, :], ps2[:], Act.Copy,
                                     accum_out=sum2[:, m:m + 1])
                nc.scalar.activation(s_t[0:cout, :], ps2[:], Act.Square,
                                     accum_out=sq2[:, m:m + 1])

            st2 = stat.tile([cout, 2], dt.float32, tag="st2")
            nc.vector.tensor_reduce(out=st2[:, 0:1], in_=sum2[:],
                                    axis=mybir.AxisListType.X, op=Alu.add)
            nc.vector.tensor_reduce(out=st2[:, 1:2], in_=sq2[:],
                                    axis=mybir.AxisListType.X, op=Alu.add)
            cc_in2 = nc.dram_tensor(f"cc_in2_{li}", [cout, 2], dt.float32)
            cc_out2 = nc.dram_tensor(f"cc_out2_{li}", [cout, 2], dt.float32,
                                     **cc_space)
            nc.sync.dma_start(out=cc_in2.ap(), in_=st2[:])
            nc.gpsimd.collective_compute(
                "AllReduce", Alu.add, replica_groups=[groups_all],
                ins=[cc_in2.ap()], outs=[cc_out2.ap()])
            gst2 = stat.tile([cout, 2], dt.float32, tag="gst2")
            nc.sync.dma_start(out=gst2[:], in_=cc_out2.ap())
            al2, be2 = _bn_affine(nc, stat, gst2, gv2, bv2, count, cout)
            al_f, be_f, c_f = al2, be2, cout
        else:
            al_f, be_f, c_f = al1, be1, cmid

        # ---------------- layer output: lrelu(alpha*kmax + beta) ----------------
        xn = keep.tile([c_f, N], dt.float32, tag="xcur")
        nc.scalar.activation(xn[:], kmax[:], Act.Identity, bias=be_f[:],
                             scale=al_f[:])
        nc.vector.scalar_tensor_tensor(out=xn[:], in0=xn[:], scalar=NEG,
                                       in1=xn[:], op0=Alu.mult, op1=Alu.max)
        nc.sync.dma_start(out=outs[L["out"]], in_=xn[:])
        x_cur = xn[:]


# ------------------------------------------------------------------
# build + host-side kernel entry
# ------------------------------------------------------------------

IN_SPECS = [("x", None)]  # x shape filled per n_points
for _i, _shp in enumerate([(64, 6), (64, 64), (64, 128), (64, 64), (128, 128)]):
    IN_SPECS.append((f"W{_i + 1}", _shp))
    IN_SPECS.append((f"g{_i + 1}", (_shp[0],)))
    IN_SPECS.append((f"b{_i + 1}", (_shp[0],)))


def build_nc(n_points=N_FULL, n_cores=B_FULL):
    nc = bacc.Bacc("TRN2", target_bir_lowering=False, debug=False,
                   num_devices=n_cores)
    ins = {}
    for name, shp in IN_SPECS:
        if name == "x":
            shp = (3, n_points)
        ins[name] = nc.dram_tensor(name, list(shp), dt.float32,
                                   kind="ExternalInput").ap()
    outs = {
        "o1": nc.dram_tensor("o1", [64, n_points], dt.float32,
                             kind="ExternalOutput").ap(),
        "o2": nc.dram_tensor("o2", [64, n_points], dt.float32,
                             kind="ExternalOutput").ap(),
        "o3": nc.dram_tensor("o3", [128, n_points], dt.float32,
                             kind="ExternalOutput").ap(),
    }
    with tile.TileContext(nc) as tc:
        with ExitStack() as ctx:
            emit_dgcnn(tc, ins, outs, n_points, n_cores, ctx)
    nc.compile()
    return nc


_NC_CACHE = {}


def run_on_hw(inputs, n_points=N_FULL, n_cores=B_FULL, trace=False):
    from concourse.bass_utils import run_bass_kernel_spmd
    key = (n_points, n_cores)
    if key not in _NC_CACHE:
        _NC_CACHE[key] = build_nc(n_points, n_cores)
    nc = _NC_CACHE[key]
    in_maps = []
    for b in range(n_cores):
        m = {"x": np.ascontiguousarray(np.asarray(inputs["x"])[b], np.float32)}
        for name, _ in IN_SPECS:
            if name != "x":
                m[name] = np.ascontiguousarray(np.asarray(inputs[name]), np.float32)
        in_maps.append(m)
    try:
        res = run_bass_kernel_spmd(nc, in_maps, list(range(n_cores)), trace=trace)
    except ModuleNotFoundError:
        res = run_bass_kernel_spmd(nc, in_maps, list(range(n_cores)), trace=False)
    x1 = np.stack([res.results[b]["o1"] for b in range(n_cores)])
    x2 = np.stack([res.results[b]["o2"] for b in range(n_cores)])
    x3 = np.stack([res.results[b]["o3"] for b in range(n_cores)])
    return (x3, [x1, x2, x3]), res


def kernel(**inputs):
    out, _ = run_on_hw(inputs)
    return out
